# revision 1
# baseline (speedup 1.0000x reference)
"""W8A8 quantized Llama MLP on 8 Trainium2 NeuronCores.

Sharding: data-parallel over tokens (8 cores x 512 tokens, no collectives).
Per core: bf16 matmuls (int8 values are exact in bf16, fp32 PSUM accumulation
is exact for these magnitudes), fp32 dequant/SiLU epilogue, per-token dynamic
requant via magic-number round-to-nearest-even, DMA-transpose of y_q for the
down-projection, fp32 dequant epilogue.

MM1 streams w_gate_up chunks once per token-pair group (2x weight reuse);
MM2 streams w_down chunks once for all 4 token subtiles (4x reuse).
The intermediate y (fp32) is staged through DRAM to keep SBUF under budget.
"""

import numpy as np
import ml_dtypes

T, H, I = 4096, 4096, 11008
N_CORES = 8
TC = T // N_CORES            # 512 tokens per core
S = TC // 128                # 4 token subtiles per core
GROUPS = [(0, 1), (2, 3)]    # MM1 weight-sharing groups
CH = 22                      # chunks over I (21*512 + 256)
K1 = H // 128                # 32 contraction tiles for MM1
KI = I // 128                # 86 contraction tiles for MM2
HCN = H // 512               # 8 output chunks for MM2
MAGIC = 12582912.0           # 1.5 * 2^23: fp32 RNE-to-integer magic constant

_bf16 = ml_dtypes.bfloat16

_prog_cache = {}


def _chunk_w(c):
    return 512 if c < CH - 1 else I - 512 * (CH - 1)


def _split_excess_waits(nc, mybir, bass_rust):
    """This walrus build allows only 1 sync-wait per instruction; hoist
    excess waits onto injected NOPs placed just before the instruction."""
    for f in nc.m.functions:
        for bb in f.blocks:
            insts = list(bb.instructions)
            out, changed = [], False
            for inst in insts:
                si = getattr(inst, "sync_info", None)
                if si is not None and si.on_wait is not None and len(si.on_wait) > 1:
                    waits = list(si.on_wait)
                    for w in waits[:-1]:
                        nop = bass_rust.InstNoOp(name=f"I-{nc.next_id()}", ins=[], outs=[])
                        nop.engine = inst.engine
                        nop.sync_info = mybir.SyncInfo(on_wait=[w], on_update=[])
                        out.append(nop)
                    inst.sync_info = mybir.SyncInfo(
                        on_wait=[waits[-1]], on_update=list(si.on_update or [])
                    )
                    changed = True
                out.append(inst)
            if changed:
                bb.instructions = out


def _build_program():
    import concourse.bass as bass
    import concourse.mybir as mybir
    import concourse.tile as tile
    import bass_rust
    from concourse.bass import ds, ts

    f32 = mybir.dt.float32
    bf = mybir.dt.bfloat16
    AF = mybir.ActivationFunctionType
    ALU = mybir.AluOpType
    X = mybir.AxisListType.X

    nc = bass.Bass()
    xT_d = nc.dram_tensor("xT", [H, TC], bf, kind="ExternalInput")
    xs_d = nc.dram_tensor("xs", [128, S], f32, kind="ExternalInput")
    wgu_d = nc.dram_tensor("wguT", [H, 2 * I], bf, kind="ExternalInput")
    sgu_d = nc.dram_tensor("sgu", [2 * I], f32, kind="ExternalInput")
    wd_d = nc.dram_tensor("wdT", [I, H], bf, kind="ExternalInput")
    swd_d = nc.dram_tensor("swd", [H], f32, kind="ExternalInput")
    out_d = nc.dram_tensor("out", [TC, H], f32, kind="ExternalOutput")
    ybuf_d = nc.dram_tensor("ybuf", [S, 128, I], f32, kind="Internal")

    with tile.TileContext(nc) as tc:
        with tc.tile_pool(name="consts", bufs=1) as consts, \
             tc.tile_pool(name="wpool", bufs=8) as wpool, \
             tc.tile_pool(name="spool", bufs=2) as spool, \
             tc.tile_pool(name="epool", bufs=2) as epool, \
             tc.tile_pool(name="qpool", bufs=3) as qpool, \
             tc.tile_pool(name="opool", bufs=3) as opool, \
             tc.tile_pool(name="psum", bufs=8, space="PSUM") as psum:

            xT_sb = consts.tile([128, K1, TC], bf)
            nc.sync.dma_start(xT_sb[:], xT_d[:].rearrange("(a p) t -> p a t", p=128))
            xs_sb = consts.tile([128, S], f32)
            nc.sync.dma_start(xs_sb[:], xs_d[:])
            m_all = consts.tile([128, S, CH], f32)
            mred = consts.tile([128, S], f32)
            s2_sb = consts.tile([128, S], f32)
            r_sb = consts.tile([128, S], f32)
            yqT = [consts.tile([128, KI, 128], bf, name=f"yqT{s}", tag=f"yqT{s}")
                   for s in range(S)]

            # ---------------- MM1 + epilogue + requant, per token-pair group
            for group in GROUPS:
                for c in range(CH):
                    cw = _chunk_w(c)
                    psG = [psum.tile([128, 512], f32, name=f"psG{tl}", tag="ps")
                           for tl in range(2)]
                    psU = [psum.tile([128, 512], f32, name=f"psU{tl}", tag="ps")
                           for tl in range(2)]
                    for ps_list, col_base in ((psG, 0), (psU, I)):
                        col0 = col_base + c * 512
                        for j in range(K1 // 4):
                            wt = wpool.tile([128, 4, cw], bf, name="wt", tag="w")
                            nc.sync.dma_start(
                                wt[:],
                                wgu_d[ds(j * 512, 512), ds(col0, cw)]
                                .rearrange("(a p) n -> p a n", p=128))
                            for kk in range(4):
                                k = 4 * j + kk
                                for tl, s in enumerate(group):
                                    nc.tensor.matmul(
                                        ps_list[tl][:, :cw],
                                        lhsT=xT_sb[:, k, ts(s, 128)],
                                        rhs=wt[:, kk, :],
                                        start=(k == 0), stop=(k == K1 - 1))
                    sgB = spool.tile([128, 512], f32, name="sgB", tag="sgB")
                    nc.sync.dma_start(
                        sgB[:, :cw],
                        sgu_d[ds(c * 512, cw)][None, :].to_broadcast((128, cw)))
                    suB = spool.tile([128, 512], f32, name="suB", tag="suB")
                    nc.sync.dma_start(
                        suB[:, :cw],
                        sgu_d[ds(I + c * 512, cw)][None, :].to_broadcast((128, cw)))
                    for tl, s in enumerate(group):
                        xs_ap = xs_sb[:, s:s + 1]
                        g_t = epool.tile([128, 512], f32, name="g_t", tag="g")
                        nc.vector.scalar_tensor_tensor(
                            g_t[:, :cw], psG[tl][:, :cw], xs_ap, sgB[:, :cw],
                            ALU.mult, ALU.mult)
                        sig = epool.tile([128, 512], f32, name="sig", tag="sig")
                        nc.scalar.activation(sig[:, :cw], g_t[:, :cw], AF.Sigmoid)
                        u_t = epool.tile([128, 512], f32, name="u_t", tag="u")
                        nc.vector.scalar_tensor_tensor(
                            u_t[:, :cw], psU[tl][:, :cw], xs_ap, suB[:, :cw],
                            ALU.mult, ALU.mult)
                        # w1 = sig*u (in place over sig), y = w1*g (in place over u)
                        nc.vector.tensor_tensor(sig[:, :cw], sig[:, :cw], u_t[:, :cw], ALU.mult)
                        nc.vector.tensor_tensor(u_t[:, :cw], sig[:, :cw], g_t[:, :cw], ALU.mult)
                        nc.vector.tensor_reduce(
                            m_all[:, s, c:c + 1], u_t[:, :cw], axis=X, op=ALU.max,
                            apply_absolute_value=True)
                        nc.gpsimd.dma_start(ybuf_d[s, :, ds(c * 512, cw)], u_t[:, :cw])

                # requant this group's subtiles (overlaps next group's matmuls)
                for s in group:
                    nc.vector.tensor_reduce(
                        mred[:, s:s + 1], m_all[:, s, :], axis=X, op=ALU.max)
                    nc.vector.tensor_scalar(
                        s2_sb[:, s:s + 1], mred[:, s:s + 1], 1e-8, 1.0 / 127.0,
                        ALU.max, ALU.mult)
                    nc.vector.reciprocal(r_sb[:, s:s + 1], s2_sb[:, s:s + 1])
                    for c in range(CH):
                        cw = _chunk_w(c)
                        ych = qpool.tile([128, 512], f32, name="ych", tag="ych")
                        nc.gpsimd.dma_start(ych[:, :cw], ybuf_d[s, :, ds(c * 512, cw)])
                        t1 = qpool.tile([128, 512], f32, name="t1", tag="t1")
                        nc.scalar.activation(t1[:, :cw], ych[:, :cw], AF.Copy,
                                             bias=MAGIC, scale=r_sb[:, s:s + 1])
                        yq = qpool.tile([128, 512], bf, name="yq", tag="yq")
                        nc.vector.tensor_scalar(yq[:, :cw], t1[:, :cw], MAGIC, None,
                                                ALU.subtract)
                        for j in range(cw // 128):
                            nc.scalar.dma_start_transpose(
                                yqT[s][:, c * 4 + j, :], yq[:, ts(j, 128)])

            # ---------------- MM2: out = dequant(y_q @ w_down.T)
            for hc in range(HCN):
                swdB = spool.tile([128, 512], f32, name="swdB", tag="swdB")
                nc.sync.dma_start(
                    swdB[:],
                    swd_d[ds(hc * 512, 512)][None, :].to_broadcast((128, 512)))
                ps2 = [psum.tile([128, 512], f32, name=f"ps2_{s}", tag="ps")
                       for s in range(S)]
                for j in range((KI + 3) // 4):
                    kn = min(4, KI - 4 * j)
                    wt2 = wpool.tile([128, kn, 512], bf, name="wt2", tag="w")
                    nc.sync.dma_start(
                        wt2[:],
                        wd_d[ds(j * 512, kn * 128), ds(hc * 512, 512)]
                        .rearrange("(a p) n -> p a n", p=128))
                    for kk in range(kn):
                        ki = 4 * j + kk
                        for s in range(S):
                            nc.tensor.matmul(
                                ps2[s][:], lhsT=yqT[s][:, ki, :], rhs=wt2[:, kk, :],
                                start=(ki == 0), stop=(ki == KI - 1))
                for s in range(S):
                    ot = opool.tile([128, 512], f32, name="ot", tag="ot")
                    nc.vector.scalar_tensor_tensor(
                        ot[:], ps2[s][:], s2_sb[:, s:s + 1], swdB[:],
                        ALU.mult, ALU.mult)
                    nc.sync.dma_start(out_d[ds(s * 128, 128), ds(hc * 512, 512)], ot[:])

    _split_excess_waits(nc, mybir, bass_rust)
    return nc


def kernel(x_q, x_scale, w_gate_up, s_w_gate_up, w_down, s_w_down):
    from concourse.bass_utils import run_bass_kernel_spmd

    if "nc" not in _prog_cache:
        _prog_cache["nc"] = _build_program()
    nc = _prog_cache["nc"]

    wguT = np.ascontiguousarray(np.asarray(w_gate_up).T).astype(np.float32).astype(_bf16)
    wdT = np.ascontiguousarray(np.asarray(w_down).T).astype(np.float32).astype(_bf16)
    sgu = np.asarray(s_w_gate_up, dtype=np.float32)
    swd = np.asarray(s_w_down, dtype=np.float32)

    in_maps = []
    for c in range(N_CORES):
        xc = np.asarray(x_q[c * TC:(c + 1) * TC, :])
        xT = np.ascontiguousarray(xc.T).astype(np.float32).astype(_bf16)
        xs = np.ascontiguousarray(
            np.asarray(x_scale[c * TC:(c + 1) * TC], dtype=np.float32)
            .reshape(S, 128).T)
        in_maps.append({"xT": xT, "xs": xs, "wguT": wguT, "sgu": sgu,
                        "wdT": wdT, "swd": swd})

    res = run_bass_kernel_spmd(nc, in_maps, core_ids=list(range(N_CORES)),
                               trace=False)
    return np.concatenate([r["out"] for r in res.results], axis=0)

